# revision 14
# baseline (speedup 1.0000x reference)
# Trainium2 Bass kernel for GQA attention block (RMSNorm -> QKV -> RoPE ->
# causal attention -> output projection), tensor-parallel over heads on 8
# NeuronCores.
#
# Design (vs the naive per-head AllGather formulation):
#  - wo is column-sharded: each core computes its local output-projection
#    partial (attnout_local @ wo[:, c-slice].T) right after each 512-seq
#    attention chunk; a per-chunk bf16 ReduceScatter(add) then lands each
#    core's 64-row output shard. The PE never waits on collectives; only
#    the last chunk's RS is exposed at the tail.
#  - Causal trimming: diagonal key-blocks run score/exp/av/den matmuls only
#    over the attending query columns (ragged PSUM accumulation,
#    narrowest-diagonal-first so start=True clears the bank).
#  - Queue discipline: bulk loads on the SP HW DMA queue; k/v weight tiles
#    and partial-out writes on the ACT HW queue; RS-dependent output copies
#    on the gpsimd SWDGE queue (except the last chunk) so no strict-FIFO
#    queue ever stalls behind a collective-dependent transfer.
#  - PSUM: 8 banks = qkv/stats ring (2) + score ring (2) + av (1, shared
#    with the V-transpose landing in the o-ring) + den (1) + wo-partial
#    ring (2).
#
# Self-contained: hardcodes all shapes; host-side prep shards/permutes the
# inputs, the device program is identical SPMD on cores 0-7, and the host
# reassembles the per-core [4, 64, 4096] row shards (cast bf16 -> f32).
import math

import numpy as np
import ml_dtypes

import concourse.bass as bass
import concourse.mybir as mybir
import concourse.tile as tile
from concourse import bacc
from concourse.bass_utils import run_bass_kernel_spmd

BF = ml_dtypes.bfloat16

SEQ = 2048
DIM = 4096
HD = 128
N_HEADS = 32
N_KV = 8
NCORES = 8
QH = N_HEADS // NCORES          # 4 q-heads per core
DQ = QH * HD                    # 512
KT = DIM // 128                 # 32 contraction tiles
SB = SEQ // 128                 # 16 seq blocks
QC = SEQ // 512                 # 4 seq chunks
ROPE_THETA = 50000.0
EPS = 1e-5
SCALE = 1.0 / math.sqrt(HD)

f32 = mybir.dt.float32
bf16 = mybir.dt.bfloat16

# stash of the last run's BassKernelResults (for test.py to read timing)
LAST_RESULT = None


def host_prep(hidden, norm_w, wq, wk, wv, wo):
    """Build the 8 per-core input maps (numpy, host-side)."""
    hidden = np.asarray(hidden, dtype=np.float32)
    norm_w = np.asarray(norm_w, dtype=np.float32)
    wq = np.asarray(wq, dtype=np.float32)
    wk = np.asarray(wk, dtype=np.float32)
    wv = np.asarray(wv, dtype=np.float32)
    wo = np.asarray(wo, dtype=np.float32)

    # rope pair permutation within a head: evens (2i) first, odds (2i+1) last
    perm = np.concatenate([np.arange(0, HD, 2), np.arange(1, HD, 2)])

    inv_freq = 1.0 / (ROPE_THETA ** (np.arange(0, HD, 2, dtype=np.float64) / HD))
    t = np.arange(SEQ, dtype=np.float64)
    ang = t[None, :] * inv_freq[:, None]            # [64, SEQ]
    cos64 = np.cos(ang)
    sin64 = np.sin(ang)
    cosb = np.concatenate([cos64, cos64], axis=0).astype(np.float32)       # [128,SEQ]
    # +sin on top half, -sin on bottom half (pre-swapped sign trick)
    sinb_pre = np.concatenate([sin64, -sin64], axis=0).astype(np.float32)  # [128,SEQ]

    maskdiag = np.triu(np.ones((128, 128), np.float32)).astype(BF)  # 1 iff p<=f
    ones128 = np.ones((128, 128), BF)
    ident = np.eye(128, dtype=BF)

    hT = np.ascontiguousarray(hidden.T).astype(BF)  # [DIM, SEQ]

    wn = norm_w[None, :]
    maps = []
    for c in range(NCORES):
        wq_c = (wq[c * DQ:(c + 1) * DQ] * wn).reshape(QH, HD, DIM)[:, perm, :]
        wq_c = wq_c.reshape(DQ, DIM)
        wk_c = (wk[c * HD:(c + 1) * HD] * wn)[perm, :]
        wv_c = wv[c * HD:(c + 1) * HD] * wn
        # column block of wo: partial_out = attnout_c @ wo[:, c-slice].T
        woT_c = np.ascontiguousarray(wo[:, c * DQ:(c + 1) * DQ].T)  # [DQ, DIM]
        maps.append({
            "hT": hT,
            "wqT": np.ascontiguousarray(wq_c.T).astype(BF),   # [DIM, DQ]
            "wkT": np.ascontiguousarray(wk_c.T).astype(BF),   # [DIM, HD]
            "wvT": np.ascontiguousarray(wv_c.T).astype(BF),   # [DIM, HD]
            "woT": woT_c.astype(BF),                          # [DQ, DIM]
            "cosb": cosb,
            "sinb_pre": sinb_pre,
            "maskdiag": np.ascontiguousarray(maskdiag),
            "ones128": ones128,
            "ident": ident,
        })
    return maps


def assemble(outs):
    """outs: list of 8 per-core arrays [QC, 64, DIM] -> full [SEQ, DIM].

    Chunks 0..2 reduce-scatter whole (64-row shards); the last chunk
    scatters in two 256-row halves (32-row shards each)."""
    full = np.empty((SEQ, DIM), np.float32)
    for c in range(NCORES):
        o = np.asarray(outs[c]).astype(np.float32)
        for qc in range(QC - 1):
            r0 = qc * 512 + c * 64
            full[r0:r0 + 64] = o[qc]
        qc = QC - 1
        for part in range(2):
            r0 = qc * 512 + part * 256 + c * 32
            full[r0:r0 + 32] = o[qc, part * 32:(part + 1) * 32]
    return full


def build_body(nc, tc, ins, out_ap):
    """Emit the Tile program. ins: dict name -> AP. out_ap: [QC, 64, DIM]."""
    HALF = SEQ // 2              # 1024

    with tc.tile_pool(name="consts", bufs=1) as consts:
        mask_sb = consts.tile([128, 128], bf16, name="mask_sb")
        nc.sync.dma_start(mask_sb[:], ins["maskdiag"])
        ones_sb = consts.tile([128, 128], bf16, name="ones_sb")
        nc.sync.dma_start(ones_sb[:], ins["ones128"][:, 0:128])
        ident_sb = consts.tile([128, 128], bf16, name="ident_sb")
        nc.sync.dma_start(ident_sb[:], ins["ident"])
        ident_f32 = consts.tile([128, 128], f32, name="ident_f32")
        nc.vector.tensor_copy(out=ident_f32[:], in_=ident_sb[:])
        # explicit bias tiles (raw builds have no preamble const APs)
        eps_f32 = consts.tile([128, 1], f32, name="eps_f32")
        nc.vector.memset(eps_f32[:], EPS)
        zero_f32 = consts.tile([128, 1], f32, name="zero_f32")
        nc.vector.memset(zero_f32[:], 0.0)
        zero_bf16 = consts.tile([128, 1], bf16, name="zero_bf16")
        nc.vector.memset(zero_bf16[:], 0.0)

        with tc.tile_pool(name="qkvout", bufs=1) as qkvout, \
             tc.tile_pool(name="dram", bufs=1, space="DRAM") as dramp:
            QT = [qkvout.tile([128, SEQ], bf16, name=f"qt{b}") for b in range(QH)]
            KTile = qkvout.tile([128, SEQ], bf16, name="ktile")
            Vn = qkvout.tile([128, SB, 128], bf16, name="vn")
            partial = dramp.tile([SEQ, DIM], bf16, name="partial",
                                 tag="partial", bufs=1)
            # RS can't write IO tensors directly; bounce through DRAM scratch
            rsout = dramp.tile([QC, SEQ // QC // NCORES, DIM], bf16,
                               name="rsout", tag="rsout", bufs=1)

            with tc.tile_pool(name="ht", bufs=1) as htp, \
                 tc.tile_pool(name="wts", bufs=1) as wtp, \
                 tc.tile_pool(name="trig", bufs=1) as trig, \
                 tc.tile_pool(name="p1tmp", bufs=1) as tmp, \
                 tc.tile_pool(name="p2", bufs=1) as p2, \
                 tc.tile_pool(name="psqkv", bufs=2, space="PSUM") as psqkv, \
                 tc.tile_pool(name="pss", bufs=2, space="PSUM") as pss, \
                 tc.tile_pool(name="psav", bufs=1, space="PSUM") as psav, \
                 tc.tile_pool(name="psden", bufs=1, space="PSUM") as psden, \
                 tc.tile_pool(name="pso", bufs=2, space="PSUM") as pso:

                cosr = trig.tile([128, SEQ], f32, name="cosr")
                sinr = trig.tile([128, SEQ], f32, name="sinr")

                wq_sb = wtp.tile([128, KT, DQ], bf16, name="wq_sb")
                wos = wtp.tile([128, QH, DIM], bf16, name="wos")
                wqT_ap = ins["wqT"].rearrange("(kt p) d -> p kt d", p=128)
                wkT_ap = ins["wkT"].rearrange("(kt p) d -> p kt d", p=128)
                wvT_ap = ins["wvT"].rearrange("(kt p) d -> p kt d", p=128)
                woT_ap = ins["woT"].rearrange("(h p) d -> p h d", p=128)
                hT_ap = ins["hT"].rearrange("(kt p) s -> p kt s", p=128)

                def attn_chunk(qc):
                    """Attention for all local heads at q-chunk qc, then the
                    local wo partial + ReduceScatter for that chunk."""
                    aos = []
                    for h in range(QH):
                        av = psav.tile([128, 512], f32, tag="av",
                                       name=f"av_{h}_{qc}")
                        den = psden.tile([128, 512], f32, tag="den",
                                         name=f"den_{h}_{qc}")
                        nkb = 4 * qc + 4
                        # diagonal key-blocks first (narrowest first) so the
                        # start=True clear happens on the first MM; ragged
                        # per-element has_written handles the rest. Columns
                        # [0:q0) of a diagonal block never attend to it, so
                        # score/exp/av/den are all trimmed to [q0:512].
                        # Software-pipelined by one: score(kb+1) is issued
                        # before av/den(kb) so the PE FIFO never stalls
                        # behind the exp(kb) semaphore.
                        kbs = [4 * qc + 3 - i for i in range(4)] + \
                              list(range(4 * qc))

                        def emit_score(ki):
                            kb = kbs[ki]
                            j = kb - 4 * qc
                            q0 = max(j, 0) * 128
                            ss = pss.tile([128, 512], f32, tag="s",
                                          name=f"ss_{h}_{qc}_{kb}")
                            nc.tensor.matmul(
                                ss[:, q0:512],
                                lhsT=KTile[:, kb * 128:(kb + 1) * 128],
                                rhs=QT[h][:, qc * 512 + q0:(qc + 1) * 512],
                                start=True, stop=True)
                            es = p2.tile([128, 512], bf16, tag="es",
                                         bufs=4, name=f"es_{h}_{qc}_{kb}")
                            nc.scalar.activation(
                                es[:, q0:512], ss[:, q0:512],
                                mybir.ActivationFunctionType.Exp,
                                bias=zero_f32[:], scale=SCALE)
                            if j >= 0:
                                nc.vector.tensor_mul(
                                    out=es[:, q0:q0 + 128],
                                    in0=es[:, q0:q0 + 128],
                                    in1=mask_sb[:])
                            return es, q0

                        es0, q00 = emit_score(0)
                        for ki in range(nkb):
                            kb = kbs[ki]
                            es_c, q0_c = es0, q00
                            if ki + 1 < nkb:
                                es0, q00 = emit_score(ki + 1)
                            nc.tensor.matmul(
                                av[:, q0_c:512], lhsT=Vn[:, kb, :],
                                rhs=es_c[:, q0_c:512],
                                start=(ki == 0), stop=(ki == nkb - 1))
                            nc.tensor.matmul(
                                den[:, q0_c:512], lhsT=ones_sb[:],
                                rhs=es_c[:, q0_c:512],
                                start=(ki == 0), stop=(ki == nkb - 1))
                        dinv = p2.tile([128, 512], f32, tag="dinv", bufs=1,
                                       name=f"dinv_{h}_{qc}")
                        nc.vector.reciprocal(dinv[:], den[:])
                        ao = p2.tile([128, 512], bf16, tag="ao", bufs=6,
                                     name=f"ao_{h}_{qc}")
                        nc.vector.tensor_mul(out=ao[:], in0=av[:], in1=dinv[:])
                        aos.append(ao)
                    # local output-projection partial for this seq chunk.
                    # The last chunk reduce-scatters in two sb-pair halves so
                    # only a half-sized RS is exposed at the very end.
                    nparts = 2 if qc == QC - 1 else 1
                    for part in range(nparts):
                        sb4s = (range(4) if nparts == 1 else
                                range(part * 2, part * 2 + 2))
                        for sb4 in sb4s:
                            for oc in range(8):
                                po = pso.tile([128, 512], f32, tag="o",
                                              name=f"po_{qc}_{sb4}_{oc}")
                                for h in range(QH):
                                    nc.tensor.matmul(
                                        po[:],
                                        lhsT=aos[h][:, sb4 * 128:(sb4 + 1) * 128],
                                        rhs=wos[:, h, oc * 512:(oc + 1) * 512],
                                        start=(h == 0), stop=(h == QH - 1))
                                ob = p2.tile([128, 512], bf16, tag="ob", bufs=3,
                                             name=f"ob_{qc}_{sb4}_{oc}")
                                nc.vector.tensor_copy(out=ob[:], in_=po[:])
                                sb = qc * 4 + sb4
                                nc.scalar.dma_start(
                                    partial[sb * 128:(sb + 1) * 128,
                                            oc * 512:(oc + 1) * 512], ob[:])
                        r0 = qc * 512 + part * (512 // nparts)
                        rows = 512 // nparts
                        s0 = part * (64 // nparts)
                        srows = 64 // nparts
                        nc.gpsimd.collective_compute(
                            "ReduceScatter", mybir.AluOpType.add,
                            replica_groups=[list(range(NCORES))],
                            ins=[partial[r0:r0 + rows, :].opt()],
                            outs=[rsout[qc, s0:s0 + srows, :].opt()])
                        # gpsimd (SWDGE) queue mid-kernel: keeps this
                        # RS-dependent copy off the HW DMA queues so loads
                        # don't stall behind it. Last chunk: HW queue (fast,
                        # nothing left to poison).
                        eng = nc.sync if qc == QC - 1 else nc.gpsimd
                        eng.dma_start(out_ap[qc, s0:s0 + srows, :],
                                      rsout[qc, s0:s0 + srows, :])

                for half in range(2):
                    h0 = half * HALF
                    hts = []
                    for kt in range(KT):
                        t = htp.tile([128, HALF], bf16, tag="ht", bufs=33,
                                     name=f"ht_{half}_{kt}")
                        nc.sync.dma_start(t[:], hT_ap[:, kt, h0:h0 + HALF])
                        hts.append(t)
                    if half == 0:
                        # trig + weights after the ht issues so compute
                        # input wins the DMA queues; trig on SP so the ACT
                        # queue stays clear for the stats squares
                        nc.sync.dma_start(cosr[:], ins["cosb"])
                        nc.sync.dma_start(sinr[:], ins["sinb_pre"])
                        for kt in range(KT):
                            nc.sync.dma_start(wq_sb[:, kt, :], wqT_ap[:, kt, :])
                        for h in range(QH):
                            nc.sync.dma_start(wos[:, h, :], woT_ap[:, h, :])

                    # --- rms stats: sumsq over dim via ACT square +
                    # ones-matmul (accumulated in the qkv psum ring)
                    Rrs = {}
                    for c2 in range(HALF // 512):
                        c0 = h0 + c2 * 512
                        ps_r = psqkv.tile([128, 512], f32, tag="qkv",
                                          name=f"ps_r_{half}_{c2}")
                        for kt in range(KT):
                            sq = tmp.tile([128, 512], bf16, tag="sq", bufs=3,
                                          name=f"sq_{half}_{c2}_{kt}")
                            nc.scalar.activation(
                                sq[:], hts[kt][:, c2 * 512:(c2 + 1) * 512],
                                mybir.ActivationFunctionType.Square,
                                bias=zero_bf16[:])
                            nc.tensor.matmul(
                                ps_r[:], lhsT=ones_sb[:], rhs=sq[:],
                                start=(kt == 0), stop=(kt == KT - 1))
                        srt = tmp.tile([128, 512], f32, tag="srt", bufs=1,
                                       name=f"srt_{half}_{c2}")
                        # sqrt(sumsq/DIM + EPS)
                        nc.scalar.activation(
                            srt[:], ps_r[:], mybir.ActivationFunctionType.Sqrt,
                            bias=eps_f32[:], scale=1.0 / DIM)
                        rr = tmp.tile([128, 512], f32, tag="rr", bufs=2,
                                      name=f"rr_{half}_{c2}")
                        nc.vector.reciprocal(rr[:], srt[:])
                        Rrs[c2] = rr
                        nc.vector.tensor_mul(
                            out=cosr[:, c0:c0 + 512], in0=cosr[:, c0:c0 + 512],
                            in1=rr[:])
                        nc.vector.tensor_mul(
                            out=sinr[:, c0:c0 + 512], in0=sinr[:, c0:c0 + 512],
                            in1=rr[:])

                    # k/v weight tiles: hoisted ring loads on the ACT DMA
                    # queue so the first projection chains aren't stuck
                    # behind ht/wq traffic on the SP queue
                    wkv_tiles = {}
                    for b, w_ap in ((4, wkT_ap), (5, wvT_ap)):
                        wkv_tiles[b] = [tmp.tile([128, 128], bf16, tag="wkv",
                                                 bufs=8,
                                                 name=f"w{b}_{half}_{kt}")
                                        for kt in range(KT)]
                        for kt in range(KT):
                            nc.scalar.dma_start(wkv_tiles[b][kt][:],
                                                w_ap[:, kt, :])

                    # --- QKV projections: blocks 0..3 = q heads, 4 = k, 5 = v
                    # K/V first so attention S-matmuls can be hoisted by the
                    # scheduler as soon as each q head completes
                    for b in (4, 5, 0, 1, 2, 3):
                        for ch in range(HALF // 512):
                            s0 = h0 + ch * 512
                            sl = slice(ch * 512, (ch + 1) * 512)
                            ps = psqkv.tile([128, 512], f32, tag="qkv",
                                            name=f"ps_{half}_{b}_{ch}")
                            for kt in range(KT):
                                if b < 4:
                                    lhsT = wq_sb[:, kt, b * 128:(b + 1) * 128]
                                else:
                                    lhsT = wkv_tiles[b][kt][:]
                                nc.tensor.matmul(
                                    ps[:], lhsT=lhsT, rhs=hts[kt][:, sl],
                                    start=(kt == 0), stop=(kt == KT - 1))
                            if b == 5:
                                # V: scale by r, transpose 128-blocks to natural
                                vsc = tmp.tile([128, 512], f32, tag="vsc",
                                               bufs=1, name=f"vsc_{half}_{ch}")
                                nc.vector.tensor_mul(
                                    out=vsc[:], in0=ps[:], in1=Rrs[ch][:])
                                # transposes land in the av psum ring (4 x 128
                                # columns of one bank) to save a PSUM bank
                                pvt = pso.tile([128, 512], f32, tag="o",
                                                name=f"pvt_{half}_{ch}")
                                for j in range(4):
                                    nc.tensor.transpose(
                                        pvt[:, j * 128:(j + 1) * 128],
                                        vsc[:, j * 128:(j + 1) * 128],
                                        ident_f32[:])
                                nc.vector.tensor_copy(
                                    out=Vn[:, s0 // 128:s0 // 128 + 4, :],
                                    in_=pvt[:])
                            else:
                                # rope: out = x*cosr + swap(x*sinr_pre)
                                dst = QT[b] if b < QH else KTile
                                m1 = tmp.tile([128, 512], f32, tag="m1", bufs=1,
                                              name=f"m1_{half}_{b}_{ch}")
                                nc.vector.tensor_mul(
                                    out=m1[:], in0=ps[:], in1=cosr[:, s0:s0 + 512])
                                m2p = tmp.tile([128, 512], f32, tag="m2p", bufs=1,
                                               name=f"m2p_{half}_{b}_{ch}")
                                nc.vector.tensor_mul(
                                    out=m2p[:], in0=ps[:], in1=sinr[:, s0:s0 + 512])
                                m2 = tmp.tile([128, 512], f32, tag="m2", bufs=1,
                                              name=f"m2_{half}_{b}_{ch}")
                                nc.sync.dma_start(m2[0:64, :], m2p[64:128, :])
                                nc.sync.dma_start(m2[64:128, :], m2p[0:64, :])
                                nc.vector.tensor_add(
                                    out=dst[:, s0:s0 + 512], in0=m1[:], in1=m2[:])

                    # attention + wo partial + RS for the q-chunks whose keys
                    # are now complete
                    attn_chunk(2 * half)
                    attn_chunk(2 * half + 1)


def build_program(reps=1):
    nc = bacc.Bacc("TRN2", target_bir_lowering=False, debug=False,
                   num_devices=NCORES)
    in_specs = {
        "hT": ([DIM, SEQ], bf16),
        "wqT": ([DIM, DQ], bf16),
        "wkT": ([DIM, HD], bf16),
        "wvT": ([DIM, HD], bf16),
        "woT": ([DQ, DIM], bf16),
        "cosb": ([128, SEQ], f32),
        "sinb_pre": ([128, SEQ], f32),
        "maskdiag": ([128, 128], bf16),
        # width encodes reps so differently-unrolled builds can't alias in
        # the jit/AOT compile cache (keys include input avals)
        "ones128": ([128, 128 + (reps - 1)], bf16),
        "ident": ([128, 128], bf16),
    }
    ins = {}
    for name, (shape, dt) in in_specs.items():
        ins[name] = nc.dram_tensor(name, shape, dt, kind="ExternalInput").ap()
    out_t = nc.dram_tensor("out", [QC, SEQ // QC // NCORES, DIM], bf16,
                           kind="ExternalOutput")
    with tile.TileContext(nc) as tc:
        for _ in range(reps):
            build_body(nc, tc, ins, out_t.ap())
    nc.compile()
    return nc


def kernel(**inputs):
    global LAST_RESULT
    maps = host_prep(**inputs)
    nc = build_program()
    res = run_bass_kernel_spmd(nc, maps, core_ids=list(range(NCORES)))
    LAST_RESULT = res
    return assemble([res.results[c]["out"] for c in range(NCORES)])
